# revision 1
# baseline (speedup 1.0000x reference)
"""NVFP4 fake-quant SwiGLU MLP on 8 Trainium2 NeuronCores.

Sharding: data-parallel over tokens for the matmuls (each core computes 1024
of the 8192 tokens end-to-end); weight *quantization* is sharded Megatron-style
(each core fake-quants 1/8 of each weight) and the quantized bf16 weights are
AllGathered. No other collective is needed: the final output is token-sharded
and concatenated on the host.

Math: fake-quant values q*sc8 are exactly representable in bf16 (q: 2 sig
bits, sc8: e4m3fn 4 sig bits), so all three matmuls run at bf16 PE peak and
the global scales 1/(gs_a*gs_w) are applied to the f32 outputs. e2m1 and
e4m3fn round-to-nearest are computed with custom DVE ops (Veltkamp splitting
for the normal ranges + magic-constant fixed-point rounds for the subnormal
ranges).
"""
import numpy as np

import concourse.bass as bass
import concourse.mybir as mybir
import concourse.tile as tile
from concourse import bacc
from concourse.bass_utils import run_bass_kernel_spmd
from concourse.dve_spec import (
    Spec, Src0, Src1, C0, C1, C2, C3, One, Zero, lower, maxx, minn, select, sq,
    _has_src1, _spill_c3_to_src1,
)
import concourse.dve_ops as dve_ops_mod
from concourse.dve_ops import DveOp, OPS
from concourse.dve_uop import DveOpSpec

F32 = mybir.dt.float32
BF16 = mybir.dt.bfloat16
ALU = mybir.AluOpType
AX = mybir.AxisListType
AF = mybir.ActivationFunctionType

B, S, H, I = 4, 2048, 1024, 4096
NCORES = 8
T = B * S
T_LOC = T // NCORES      # 1024 tokens per core
I_SH = I // NCORES       # 512  gate/up rows per core (quant shard)
HO_SH = H // NCORES      # 128  down rows per core (quant shard)

VELT_E2M1 = float(2**22 + 1)
MAGIC_E2M1 = float(3 * 2**21)
VELT_E4M3 = float(2**20 + 1)
MAGIC_E4M3 = float(2**14)
TH_E4M3 = float(2**-6)

# ---------------------------------------------------------------- custom ops


def _register(name, spec, subdim=False):
    for op in OPS:
        if op.name == name:
            return op
    idx = len(OPS)
    opcode = dve_ops_mod._CUSTOM_DVE_ROW_BASE + idx
    assert opcode < 0x20, "custom DVE row overflow"
    shas = {}
    for ver in ("v3", "v4"):
        shas[ver] = DveOpSpec(
            name=name, opcode=opcode, uops=lower(spec, ver=ver),
            rd1_en=_has_src1(spec),
        ).sha(ver)
    op = DveOp(name, spec, subdim=subdim, uops_sha=shas)
    OPS.append(op)
    dve_ops_mod._SUB_OPCODE_FOR_NAME[name] = opcode
    dve_ops_mod.CUSTOM_DVE_SPECS[name] = spec
    return op


def _ref_scale_clip(in0, in1, s0, s1, imm2):
    m = (in0.astype(np.float32) * in1.astype(np.float32)).astype(np.float32)
    return np.minimum(np.maximum(m, np.float32(-s0)), np.float32(s0))


def _ref_subnorm_sel(in0, in1, s0, s1, imm2):
    t = in0.astype(np.float32)
    u = (t + np.float32(s0)).astype(np.float32)
    v = (u - np.float32(s0)).astype(np.float32)
    return np.where((t * t).astype(np.float32) < 1.0, v, t).astype(np.float32)


def _ref_velt_scale(in0, in1, s0, s1, imm2):
    t = in0.astype(np.float32)
    gam = (t * np.float32(s0)).astype(np.float32)
    delta = (t - gam).astype(np.float32)
    hi = (gam + delta).astype(np.float32)
    return (hi * in1.astype(np.float32)).astype(np.float32)


def _ref_e4m3(in0, in1, s0, s1, imm2):
    cap = in1.reshape(in1.shape[0], 1).astype(np.float32)
    t = np.minimum(in0.astype(np.float32), cap)
    gam = (t * np.float32(s0)).astype(np.float32)
    delta = (t - gam).astype(np.float32)
    hi = (gam + delta).astype(np.float32)
    u = (t + np.float32(s1)).astype(np.float32)
    v = (u - np.float32(s1)).astype(np.float32)
    return np.where(t < np.float32(imm2), v, hi).astype(np.float32)


_m = Src0 * Src1
OP_SCALE_CLIP = _register(
    "NVFP4_SCALE_CLIP_ANT",
    Spec(body=minn(maxx(_m, Zero - C0), C0), reference=_ref_scale_clip),
)
_u = Src0 + C0
_v = _u - C0
OP_E2M1_SUBNORM = _register(
    "NVFP4_E2M1_SUBNORM_ANT",
    Spec(body=select(sq(Src0) < One, _v, Src0), reference=_ref_subnorm_sel),
)
_gam = Src0 * C0
_hi = _gam + (Src0 - _gam)
OP_VELT_SCALE = _register(
    "NVFP4_VELT_SCALE_ANT",
    Spec(body=_hi * Src1, reference=_ref_velt_scale),
)
_t = minn(Src0, C3)
_gam4 = _t * C0
_hi4 = _gam4 + (_t - _gam4)
_v4 = (_t + C1) - C1
OP_E4M3 = _register(
    "NVFP4_E4M3_ANT",
    Spec(body=_spill_c3_to_src1(select(_t < C2, _v4, _hi4)), reference=_ref_e4m3),
)


def quantize_tile(nc, work, src_f32, out_bf16, n, gs, c448_col):
    """src_f32 [128, n] (true values, 16-blocks on free dim) -> out_bf16 = q*sc8."""
    nblk = n // 16
    gs = float(np.float32(gs))
    src3 = src_f32.rearrange("p (b s) -> p b s", s=16)
    amax = work.tile([128, nblk], F32, tag="q_amax")
    nc.vector.tensor_reduce(
        out=amax[:], in_=src3, axis=AX.X, op=ALU.max, apply_absolute_value=True
    )
    t1 = work.tile([128, nblk], F32, tag="q_t1")
    nc.vector.tensor_scalar(
        out=t1[:], in0=amax[:], scalar1=float(np.float32(1.0 / 6.0)), scalar2=gs,
        op0=ALU.mult, op1=ALU.mult,
    )
    sc8 = work.tile([128, nblk], F32, tag="q_sc8")
    nc.vector._custom_dve(
        OP_E4M3, out=sc8[:], in0=t1[:], in1=c448_col,
        s0=VELT_E4M3, s1=MAGIC_E4M3, imm2=TH_E4M3,
    )
    r = work.tile([128, nblk], F32, tag="q_r")
    nc.vector.reciprocal(r[:], sc8[:])
    r2 = work.tile([128, nblk], F32, tag="q_r2")
    nc.vector.tensor_scalar(
        out=r2[:], in0=r[:], scalar1=gs, scalar2=1e38,
        op0=ALU.mult, op1=ALU.min,
    )
    cc = work.tile([128, n], F32, tag="q_cc")
    cc3 = cc[:].rearrange("p (b s) -> p b s", s=16)
    r2b = r2[:].unsqueeze(-1).broadcast_to([128, nblk, 16])
    nc.vector._custom_dve(OP_SCALE_CLIP, out=cc3, in0=src3, in1=r2b, s0=6.0)
    pp = work.tile([128, n], F32, tag="q_pp")
    nc.vector._custom_dve(OP_E2M1_SUBNORM, out=pp[:], in0=cc[:], s0=MAGIC_E2M1)
    sc8b = sc8[:].unsqueeze(-1).broadcast_to([128, nblk, 16])
    out3 = out_bf16.rearrange("p (b s) -> p b s", s=16)
    pp3 = pp[:].rearrange("p (b s) -> p b s", s=16)
    nc.vector._custom_dve(OP_VELT_SCALE, out=out3, in0=pp3, in1=sc8b, s0=VELT_E2M1)


# ---------------------------------------------------------------- program


def build_program(gs_x, gs_gw, gs_uw, gs_dw, gs_h):
    gs_x, gs_gw, gs_uw, gs_dw, gs_h = (
        np.float32(gs_x), np.float32(gs_gw), np.float32(gs_uw),
        np.float32(gs_dw), np.float32(gs_h),
    )
    s_gate = float(np.float32(1.0) / np.float32(gs_x * gs_gw))
    s_up = float(np.float32(1.0) / np.float32(gs_x * gs_uw))
    s_down = float(np.float32(1.0) / np.float32(gs_h * gs_dw))

    nc = bacc.Bacc("TRN2", num_devices=NCORES, debug=False)
    x_in = nc.dram_tensor("x_slice", [T_LOC, H], F32, kind="ExternalInput")
    gw_in = nc.dram_tensor("gw_slice", [I_SH, H], F32, kind="ExternalInput")
    uw_in = nc.dram_tensor("uw_slice", [I_SH, H], F32, kind="ExternalInput")
    dw_in = nc.dram_tensor("dw_slice", [HO_SH, I], F32, kind="ExternalInput")
    out_d = nc.dram_tensor("out_slice", [T_LOC, H], F32, kind="ExternalOutput")

    RG = [list(range(NCORES))]

    with tile.TileContext(nc) as tc:
        with (
            tc.tile_pool(name="dram", bufs=1, space="DRAM") as dpool,
            tc.tile_pool(name="const", bufs=1) as cpool,
            tc.tile_pool(name="xt", bufs=1) as xtpool,
        ):
            gwq_loc = dpool.tile([I_SH, H], BF16)
            uwq_loc = dpool.tile([I_SH, H], BF16)
            dwq_loc = dpool.tile([HO_SH, I], BF16)
            gwq_g = dpool.tile([I, H], BF16, addr_space="Shared")
            uwq_g = dpool.tile([I, H], BF16, addr_space="Shared")
            dwq_g = dpool.tile([H, I], BF16, addr_space="Shared")
            xq_d = dpool.tile([T_LOC, H], BF16)
            hq_d = dpool.tile([T_LOC, I], BF16)

            c448 = cpool.tile([128, 1], F32)
            nc.vector.memset(c448[:], 448.0)

            # xqsT[h-tile][128h, tok]  (resident through phase C)
            xqsT = xtpool.tile([128, H // 128, T_LOC], BF16)

            # ---- Phase A: quantize own weight shards, allgather bf16
            with (
                tc.tile_pool(name="wraw", bufs=2) as wraw,
                tc.tile_pool(name="wq", bufs=2) as wqp,
                tc.tile_pool(name="workA", bufs=2) as workA,
            ):
                for src, dst, rows, cols, gsw in (
                    (gw_in, gwq_loc, I_SH, H, gs_gw),
                    (uw_in, uwq_loc, I_SH, H, gs_uw),
                    (dw_in, dwq_loc, HO_SH, I, gs_dw),
                ):
                    for r0 in range(0, rows, 128):
                        wt = wraw.tile([128, cols], F32, tag="wraw")
                        nc.sync.dma_start(wt[:], src[r0:r0 + 128, :])
                        wq = wqp.tile([128, cols], BF16, tag="wq")
                        quantize_tile(nc, workA, wt[:], wq[:], cols, gsw, c448[:])
                        nc.sync.dma_start(dst[r0:r0 + 128, :], wq[:])

            for loc, gat in ((gwq_loc, gwq_g), (uwq_loc, uwq_g), (dwq_loc, dwq_g)):
                nc.gpsimd.collective_compute(
                    "AllGather", ALU.bypass, replica_groups=RG,
                    ins=[loc[:]], outs=[gat[:]],
                )

            # ---- Phase B: quantize x slice, build xqsT via DMA transpose
            with (
                tc.tile_pool(name="xraw", bufs=2) as xraw,
                tc.tile_pool(name="xq", bufs=2) as xqp,
                tc.tile_pool(name="workB", bufs=2) as workB,
            ):
                for tch in range(T_LOC // 128):
                    xt = xraw.tile([128, H], F32, tag="xraw")
                    nc.sync.dma_start(xt[:], x_in[tch * 128:(tch + 1) * 128, :])
                    xq = xqp.tile([128, H], BF16, tag="xq")
                    quantize_tile(nc, workB, xt[:], xq[:], H, gs_x, c448[:])
                    nc.sync.dma_start(xq_d[tch * 128:(tch + 1) * 128, :], xq[:])
                for ht in range(H // 128):
                    nc.sync.dma_start_transpose(
                        xqsT[:, ht, :], xq_d[:, ht * 128:(ht + 1) * 128]
                    )

            # ---- Phase C: gate/up matmuls + SwiGLU + hidden quant
            with (
                tc.tile_pool(name="wstr", bufs=2) as wstr,
                tc.tile_pool(name="psgu", bufs=2, space="PSUM") as psgu,
                tc.tile_pool(name="hwork", bufs=2) as hwork,
                tc.tile_pool(name="workC", bufs=2) as workC,
                tc.tile_pool(name="hqout", bufs=2) as hqout,
            ):
                for ib in range(I // 512):
                    gwT = wstr.tile([128, H // 128, 512], BF16, tag="gwT")
                    uwT = wstr.tile([128, H // 128, 512], BF16, tag="uwT")
                    for ht in range(H // 128):
                        nc.sync.dma_start_transpose(
                            gwT[:, ht, :],
                            gwq_g[ib * 512:(ib + 1) * 512, ht * 128:(ht + 1) * 128],
                        )
                        nc.sync.dma_start_transpose(
                            uwT[:, ht, :],
                            uwq_g[ib * 512:(ib + 1) * 512, ht * 128:(ht + 1) * 128],
                        )
                    for tch in range(T_LOC // 128):
                        pg = psgu.tile([128, 512], F32, tag="pg")
                        pu = psgu.tile([128, 512], F32, tag="pu")
                        for ht in range(H // 128):
                            lhsT = xqsT[:, ht, tch * 128:(tch + 1) * 128]
                            nc.tensor.matmul(
                                pg[:], lhsT, gwT[:, ht, :],
                                start=(ht == 0), stop=(ht == H // 128 - 1),
                            )
                            nc.tensor.matmul(
                                pu[:], lhsT, uwT[:, ht, :],
                                start=(ht == 0), stop=(ht == H // 128 - 1),
                            )
                        sil = hwork.tile([128, 512], F32, tag="sil")
                        nc.scalar.activation(sil[:], pg[:], AF.Silu, scale=s_gate)
                        htr = hwork.tile([128, 512], F32, tag="htr")
                        nc.vector.scalar_tensor_tensor(
                            out=htr[:], in0=sil[:], scalar=s_up, in1=pu[:],
                            op0=ALU.mult, op1=ALU.mult,
                        )
                        hq = hqout.tile([128, 512], BF16, tag="hq")
                        quantize_tile(nc, workC, htr[:], hq[:], 512, gs_h, c448[:])
                        nc.sync.dma_start(
                            hq_d[tch * 128:(tch + 1) * 128, ib * 512:(ib + 1) * 512],
                            hq[:],
                        )

            # ---- Phase D: down matmul + output scale
            with (
                tc.tile_pool(name="dwt", bufs=1) as dwtp,
                tc.tile_pool(name="hqt", bufs=2) as hqtp,
                tc.tile_pool(name="pso", bufs=2, space="PSUM") as pso,
                tc.tile_pool(name="obuf", bufs=2) as obuf,
            ):
                dwT = dwtp.tile([128, I // 128, H], BF16)
                for it in range(I // 128):
                    nc.sync.dma_start_transpose(
                        dwT[:, it, :], dwq_g[:, it * 128:(it + 1) * 128]
                    )
                for tch in range(T_LOC // 128):
                    hqT = hqtp.tile([128, I // 128, 128], BF16, tag="hqT")
                    for it in range(I // 128):
                        nc.sync.dma_start_transpose(
                            hqT[:, it, :],
                            hq_d[tch * 128:(tch + 1) * 128, it * 128:(it + 1) * 128],
                        )
                    po = pso.tile([128, H], F32, tag="po")
                    for it in range(I // 128):
                        lhsT = hqT[:, it, :]
                        nc.tensor.matmul(
                            po[:, 0:512], lhsT, dwT[:, it, 0:512],
                            start=(it == 0), stop=(it == I // 128 - 1),
                        )
                        nc.tensor.matmul(
                            po[:, 512:1024], lhsT, dwT[:, it, 512:1024],
                            start=(it == 0), stop=(it == I // 128 - 1),
                        )
                    ob = obuf.tile([128, H], F32, tag="ob")
                    nc.scalar.activation(ob[:], po[:], AF.Copy, scale=s_down)
                    nc.sync.dma_start(
                        out_d[tch * 128:(tch + 1) * 128, :], ob[:]
                    )

    nc.finalize()
    return nc


_PROG_CACHE = {}
TRACE = False          # set by test.py to capture an NTFF profile
LAST_EXEC_NS = None
LAST_RESULTS = None


def kernel(x, gate_w, up_w, down_w, s_in, s_in_down):
    x = np.ascontiguousarray(x, dtype=np.float32)
    gate_w = np.ascontiguousarray(gate_w, dtype=np.float32)
    up_w = np.ascontiguousarray(up_w, dtype=np.float32)
    down_w = np.ascontiguousarray(down_w, dtype=np.float32)
    gs_x = np.float32(np.asarray(s_in).reshape(-1)[0])
    gs_h = np.float32(np.asarray(s_in_down).reshape(-1)[0])
    FM = np.float32(448.0 * 6.0)
    gs_gw = np.float32(FM / np.abs(gate_w).max())
    gs_uw = np.float32(FM / np.abs(up_w).max())
    gs_dw = np.float32(FM / np.abs(down_w).max())

    key = tuple(float(v) for v in (gs_x, gs_gw, gs_uw, gs_dw, gs_h))
    if key not in _PROG_CACHE:
        _PROG_CACHE.clear()
        _PROG_CACHE[key] = build_program(*key)
    nc = _PROG_CACHE[key]

    xf = x.reshape(T, H)
    in_maps = []
    for c in range(NCORES):
        in_maps.append({
            "x_slice": np.ascontiguousarray(xf[c * T_LOC:(c + 1) * T_LOC]),
            "gw_slice": np.ascontiguousarray(gate_w[c * I_SH:(c + 1) * I_SH]),
            "uw_slice": np.ascontiguousarray(up_w[c * I_SH:(c + 1) * I_SH]),
            "dw_slice": np.ascontiguousarray(down_w[c * HO_SH:(c + 1) * HO_SH]),
        })
    global LAST_EXEC_NS, LAST_RESULTS
    res = run_bass_kernel_spmd(
        nc, in_maps, core_ids=list(range(NCORES)), trace=TRACE
    )
    LAST_EXEC_NS = res.exec_time_ns
    LAST_RESULTS = res
    out = np.concatenate([r["out_slice"] for r in res.results], axis=0)
    return out.reshape(B, S, H).astype(np.float32)


if __name__ == "__main__":
    rng = np.random.default_rng(0)
    inputs = dict(
        x=rng.standard_normal((B, S, H), dtype=np.float32),
        gate_w=0.05 * rng.standard_normal((I, H), dtype=np.float32),
        up_w=0.05 * rng.standard_normal((I, H), dtype=np.float32),
        down_w=0.05 * rng.standard_normal((H, I), dtype=np.float32),
        s_in=np.array([700.0], dtype=np.float32),
        s_in_down=np.array([800.0], dtype=np.float32),
    )
    out = kernel(**inputs)
    print("kernel output", out.shape, out.dtype, np.abs(out).max())



# revision 11
# speedup vs baseline: 1.5261x; 1.5261x over previous
"""NVFP4 fake-quant SwiGLU MLP on 8 Trainium2 NeuronCores.

Sharding: data-parallel over tokens (each core computes 1024 of the 8192
tokens end-to-end). Weight quantization is sharded Megatron-style: each core
fake-quants + transposes 1/8 of each weight; the quantized f16 transposed
weights are AllGathered (split in halves so compute can start after half the
bytes land). Global weight scales (448*6/max|w|) are computed on device via a
tiny AllReduce-max, so the program is input-independent and compiles once.

Math: fake-quant values q*sc8 are exactly representable in f16 (q: 2 sig
bits, sc8: e4m3fn 4 sig bits), so all three matmuls run at f16 PE peak and
the global scales 1/(gs_a*gs_w) are applied to the f32 PSUM outputs. e2m1 and
e4m3fn round-to-nearest use custom DVE ops (Veltkamp splitting for normal
ranges + magic-constant fixed-point rounds for subnormal ranges). All
transposes are PE transposes (identity matmul) - no DMA transposes.

The I dimension is processed in two halves (k=0,1); half k covers columns
[c*512 + k*256, c*512 + k*256 + 256) of every rank c's shard. The down
weight is stored in DRAM with rows permuted to slot order
slot = k*16 + c*2 + jt  <->  i-tile a = c*4 + k*2 + jt,
so the hidden transpose slots and the down matmul rhs agree.
"""
import numpy as np

import concourse.bass as bass
import concourse.bass_isa as bass_isa
import concourse.mybir as mybir
import concourse.tile as tile
from concourse import bacc
from concourse._compat import axon_active
from concourse.bass_utils import run_bass_kernel_spmd
from concourse.dve_spec import (
    Spec, Src0, Src1, C0, C1, C2, C3, One, Zero, lower, maxx, minn, select, sq,
    _has_src1, _spill_c3_to_src1,
)
import concourse.dve_ops as dve_ops_mod
from concourse.dve_ops import DveOp, OPS
from concourse.dve_uop import DveOpSpec
from concourse.masks import make_identity

F32 = mybir.dt.float32
F16 = mybir.dt.float16
ALU = mybir.AluOpType
AX = mybir.AxisListType
AF = mybir.ActivationFunctionType

B, S, H, I = 4, 2048, 1024, 4096
NCORES = 8
T = B * S                # 8192 tokens
T_LOC = T // NCORES      # 1024 tokens per core
I_SH = I // NCORES       # 512 gate/up rows per core (quant shard)
HO_SH = H // NCORES      # 128 down rows per core (quant shard)
HC = H // 128            # 8 h tiles
ION = I // 128           # 32 i tiles
TCH = T_LOC // 128       # 8 token chunks per core
IHALF = I // 2           # 2048 i columns per half

VELT_E2M1 = float(2**22 + 1)
MAGIC_E2M1 = float(3 * 2**21)
VELT_E4M3 = float(2**20 + 1)
MAGIC_E4M3 = float(2**14)
TH_E4M3 = float(2**-6)

# ---------------------------------------------------------------- custom ops


def _register(name, spec, subdim=False):
    for op in OPS:
        if op.name == name:
            return op
    idx = len(OPS)
    opcode = dve_ops_mod._CUSTOM_DVE_ROW_BASE + idx
    assert opcode < 0x20, "custom DVE row overflow"
    shas = {}
    for ver in ("v3", "v4"):
        shas[ver] = DveOpSpec(
            name=name, opcode=opcode, uops=lower(spec, ver=ver),
            rd1_en=_has_src1(spec),
        ).sha(ver)
    op = DveOp(name, spec, subdim=subdim, uops_sha=shas)
    OPS.append(op)
    dve_ops_mod._SUB_OPCODE_FOR_NAME[name] = opcode
    dve_ops_mod.CUSTOM_DVE_SPECS[name] = spec
    return op


def _ref_scale_clip(in0, in1, s0, s1, imm2):
    m = (in0.astype(np.float32) * in1.astype(np.float32)).astype(np.float32)
    return np.minimum(np.maximum(m, np.float32(-s0)), np.float32(s0))


def _ref_subnorm_sel(in0, in1, s0, s1, imm2):
    t = in0.astype(np.float32)
    u = (t + np.float32(s0)).astype(np.float32)
    v = (u - np.float32(s0)).astype(np.float32)
    return np.where((t * t).astype(np.float32) < 1.0, v, t).astype(np.float32)


def _ref_velt_scale(in0, in1, s0, s1, imm2):
    t = in0.astype(np.float32)
    gam = (t * np.float32(s0)).astype(np.float32)
    delta = (t - gam).astype(np.float32)
    hi = (gam + delta).astype(np.float32)
    return (hi * in1.astype(np.float32)).astype(np.float32)


def _ref_e4m3(in0, in1, s0, s1, imm2):
    cap = in1.reshape(in1.shape[0], 1).astype(np.float32)
    t = np.minimum(in0.astype(np.float32), cap)
    gam = (t * np.float32(s0)).astype(np.float32)
    delta = (t - gam).astype(np.float32)
    hi = (gam + delta).astype(np.float32)
    u = (t + np.float32(s1)).astype(np.float32)
    v = (u - np.float32(s1)).astype(np.float32)
    return np.where(t < np.float32(imm2), v, hi).astype(np.float32)


_m = Src0 * Src1
OP_SCALE_CLIP = _register(
    "NVFP4_SCALE_CLIP_ANT",
    Spec(body=minn(maxx(_m, Zero - C0), C0), reference=_ref_scale_clip),
)
_u = Src0 + C0
_v = _u - C0
OP_E2M1_SUBNORM = _register(
    "NVFP4_E2M1_SUBNORM_ANT",
    Spec(body=select(sq(Src0) < One, _v, Src0), reference=_ref_subnorm_sel),
)
_gam = Src0 * C0
_hi = _gam + (Src0 - _gam)
OP_VELT_SCALE = _register(
    "NVFP4_VELT_SCALE_ANT",
    Spec(body=_hi * Src1, reference=_ref_velt_scale),
)
_t = minn(Src0, C3)
_gam4 = _t * C0
_hi4 = _gam4 + (_t - _gam4)
_v4 = (_t + C1) - C1
OP_E4M3 = _register(
    "NVFP4_E4M3_ANT",
    Spec(body=_spill_c3_to_src1(select(_t < C2, _v4, _hi4)), reference=_ref_e4m3),
)


def quantize_tile(nc, work, src_f32, out_f16, n, gs_ap, gs6_ap, c448, amax_ap=None):
    """src_f32 [128, n] (true values, 16-blocks on free dim) -> out_f16 = q*sc8.

    gs_ap/gs6_ap: [128,1] APs holding the global scale and global scale / 6.
    amax_ap: optional precomputed [128, n//16] block abs-max.
    """
    nblk = n // 16
    src3 = src_f32.rearrange("p (b s) -> p b s", s=16)
    if amax_ap is None:
        amax = work.tile([128, nblk], F32, tag="q_amax")
        nc.vector.tensor_reduce(
            out=amax[:], in_=src3, axis=AX.X, op=ALU.max, apply_absolute_value=True
        )
        amax_ap = amax[:]
    t1 = work.tile([128, nblk], F32, tag="q_t1")
    nc.vector.tensor_scalar(
        out=t1[:], in0=amax_ap, scalar1=gs6_ap, scalar2=None, op0=ALU.mult,
    )
    sc8 = work.tile([128, nblk], F32, tag="q_sc8")
    nc.vector._custom_dve(
        OP_E4M3, out=sc8[:], in0=t1[:], in1=c448,
        s0=VELT_E4M3, s1=MAGIC_E4M3, imm2=TH_E4M3,
    )
    r = work.tile([128, nblk], F32, tag="q_r")
    nc.vector.reciprocal(r[:], sc8[:])
    r2 = work.tile([128, nblk], F32, tag="q_r2")
    nc.vector.tensor_scalar(
        out=r2[:], in0=r[:], scalar1=gs_ap, scalar2=1e38,
        op0=ALU.mult, op1=ALU.min,
    )
    cc = work.tile([128, n], F32, tag="q_cc")
    cc3 = cc[:].rearrange("p (b s) -> p b s", s=16)
    r2b = r2[:].unsqueeze(-1).broadcast_to([128, nblk, 16])
    nc.vector._custom_dve(OP_SCALE_CLIP, out=cc3, in0=src3, in1=r2b, s0=6.0)
    pp = work.tile([128, n], F32, tag="q_pp")
    nc.vector._custom_dve(OP_E2M1_SUBNORM, out=pp[:], in0=cc[:], s0=MAGIC_E2M1)
    sc8b = sc8[:].unsqueeze(-1).broadcast_to([128, nblk, 16])
    out3 = out_f16.rearrange("p (b s) -> p b s", s=16)
    pp3 = pp[:].rearrange("p (b s) -> p b s", s=16)
    nc.vector._custom_dve(OP_VELT_SCALE, out=out3, in0=pp3, in1=sc8b, s0=VELT_E2M1)


# ---------------------------------------------------------------- program


def build_program():
    nc = bacc.Bacc("TRN2", num_devices=NCORES, debug=False)
    x_in = nc.dram_tensor("x_slice", [T_LOC, H], F32, kind="ExternalInput")
    gw_in = nc.dram_tensor("gw_slice", [I_SH, H], F32, kind="ExternalInput")
    uw_in = nc.dram_tensor("uw_slice", [I_SH, H], F32, kind="ExternalInput")
    dw_in = nc.dram_tensor("dw_slice", [HO_SH, I], F32, kind="ExternalInput")
    sc_in = nc.dram_tensor("sc_in", [1, 2], F32, kind="ExternalInput")
    out_d = nc.dram_tensor("out_slice", [T_LOC, H], F32, kind="ExternalOutput")

    RG = [list(range(NCORES))]

    with tile.TileContext(nc) as tc:
        with (
            tc.tile_pool(name="dram", bufs=1, space="DRAM") as dpool,
            tc.tile_pool(name="const", bufs=1) as cpool,
            tc.tile_pool(name="scl", bufs=1) as spool,
        ):
            # DRAM staging for collectives (k halves kept separate for AG split)
            gwT_loc = [dpool.tile([H, I_SH // 2], F16, name=f"gwT_loc{k}") for k in range(2)]
            uwT_loc = [dpool.tile([H, I_SH // 2], F16, name=f"uwT_loc{k}") for k in range(2)]
            dwT_loc = dpool.tile([I, HO_SH], F16)      # rows in slot order
            gwT_g = [dpool.tile([NCORES * H, I_SH // 2], F16, addr_space="Shared",
                               name=f"gwT_g{k}") for k in range(2)]
            uwT_g = [dpool.tile([NCORES * H, I_SH // 2], F16, addr_space="Shared",
                               name=f"uwT_g{k}") for k in range(2)]
            dwT_g = dpool.tile([NCORES * I, HO_SH], F16, addr_space="Shared")
            wmax_l = dpool.tile([1, 4], F32)
            wmax_g = dpool.tile([1, 4], F32, addr_space="Shared")

            c448 = cpool.tile([128, 1], F32)
            nc.vector.memset(c448[:], 448.0)
            ident = cpool.tile([128, 128], F16)
            make_identity(nc, ident[:])

            # ---- input scales -> [128,1] broadcast APs
            sin_sb = spool.tile([1, 2], F32)
            nc.sync.dma_start(sin_sb[:], sc_in[:, :])
            gsx_b = spool.tile([128, 2], F32)       # cols: gs_x, gs_h
            nc.gpsimd.partition_broadcast(gsx_b[:], sin_sb[:], channels=128)
            gsx6_b = spool.tile([128, 2], F32)
            nc.vector.tensor_scalar(
                out=gsx6_b[:], in0=gsx_b[:], scalar1=float(np.float32(1.0 / 6.0)),
                scalar2=None, op0=ALU.mult,
            )
            gs_x, gs_h = gsx_b[:, 0:1], gsx_b[:, 1:2]
            gs_x6, gs_h6 = gsx6_b[:, 0:1], gsx6_b[:, 1:2]

            with (
                tc.tile_pool(name="xqT", bufs=1) as xqTp,
                tc.tile_pool(name="hqTp", bufs=1) as hqTp,
            ):
                xqT = xqTp.tile([128, HC, T_LOC], F16)
                hqT = hqTp.tile([128, ION, T_LOC], F16)

                # ============ Phase A: weight load + amax + AllReduce max
                with (
                    tc.tile_pool(name="wraw", bufs=1) as wraw,
                    tc.tile_pool(name="wam", bufs=1) as wam,
                    tc.tile_pool(name="wqt", bufs=2) as wqt,
                    tc.tile_pool(name="wtr", bufs=1) as wtr,
                    tc.tile_pool(name="pst_a", bufs=2, space="PSUM") as pst_a,
                    tc.tile_pool(name="workA", bufs=2) as workA,
                ):
                    gw_raw = [wraw.tile([128, H], F32, tag=f"gw{k}", name=f"gw_raw{k}") for k in range(4)]
                    uw_raw = [wraw.tile([128, H], F32, tag=f"uw{k}", name=f"uw_raw{k}") for k in range(4)]
                    dw_raw = wraw.tile([128, I], F32, tag="dw")
                    for k in range(4):
                        nc.sync.dma_start(gw_raw[k][:], gw_in[k * 128:(k + 1) * 128, :])
                    for k in range(4):
                        nc.sync.dma_start(uw_raw[k][:], uw_in[k * 128:(k + 1) * 128, :])
                    nc.sync.dma_start(dw_raw[:], dw_in[:, :])

                    gw_am = wam.tile([128, 4, H // 16], F32)
                    uw_am = wam.tile([128, 4, H // 16], F32)
                    dw_am = wam.tile([128, I // 16], F32)
                    for k in range(4):
                        nc.vector.tensor_reduce(
                            out=gw_am[:, k, :],
                            in_=gw_raw[k][:].rearrange("p (b s) -> p b s", s=16),
                            axis=AX.X, op=ALU.max, apply_absolute_value=True,
                        )
                    for k in range(4):
                        nc.vector.tensor_reduce(
                            out=uw_am[:, k, :],
                            in_=uw_raw[k][:].rearrange("p (b s) -> p b s", s=16),
                            axis=AX.X, op=ALU.max, apply_absolute_value=True,
                        )
                    nc.vector.tensor_reduce(
                        out=dw_am[:],
                        in_=dw_raw[:].rearrange("p (b s) -> p b s", s=16),
                        axis=AX.X, op=ALU.max, apply_absolute_value=True,
                    )
                    mx = wam.tile([128, 4], F32)
                    nc.vector.tensor_reduce(
                        out=mx[:, 0:1], in_=gw_am[:], axis=AX.XY, op=ALU.max)
                    nc.vector.tensor_reduce(
                        out=mx[:, 1:2], in_=uw_am[:], axis=AX.XY, op=ALU.max)
                    nc.vector.tensor_reduce(
                        out=mx[:, 2:3], in_=dw_am[:], axis=AX.X, op=ALU.max)
                    nc.vector.memset(mx[:, 3:4], 0.0)
                    mx_all = wam.tile([128, 4], F32)
                    nc.gpsimd.partition_all_reduce(
                        mx_all[:], mx[:], channels=128,
                        reduce_op=bass_isa.ReduceOp.max)
                    nc.sync.dma_start(wmax_l[:, :], mx_all[0:1, :])
                    nc.gpsimd.collective_compute(
                        "AllReduce", ALU.max, replica_groups=RG,
                        ins=[wmax_l[:]], outs=[wmax_g[:]],
                    )
                    wmx_sb = spool.tile([1, 4], F32)
                    nc.sync.dma_start(wmx_sb[:], wmax_g[:, :])
                    wmx_b = spool.tile([128, 4], F32)
                    nc.gpsimd.partition_broadcast(wmx_b[:], wmx_sb[:], channels=128)
                    inv_b = spool.tile([128, 4], F32)
                    nc.vector.reciprocal(inv_b[:], wmx_b[:])
                    gsw_b = spool.tile([128, 4], F32)   # cols: gs_gw, gs_uw, gs_dw
                    nc.vector.tensor_scalar(
                        out=gsw_b[:], in0=inv_b[:],
                        scalar1=float(np.float32(448.0 * 6.0)),
                        scalar2=None, op0=ALU.mult,
                    )
                    gsw6_b = spool.tile([128, 4], F32)
                    nc.vector.tensor_scalar(
                        out=gsw6_b[:], in0=gsw_b[:],
                        scalar1=float(np.float32(1.0 / 6.0)),
                        scalar2=None, op0=ALU.mult,
                    )
                    prod = spool.tile([128, 4], F32)
                    nc.vector.tensor_scalar(
                        out=prod[:, 0:2], in0=gsw_b[:, 0:2], scalar1=gs_x,
                        scalar2=None, op0=ALU.mult,
                    )
                    nc.vector.tensor_scalar(
                        out=prod[:, 2:3], in0=gsw_b[:, 2:3], scalar1=gs_h,
                        scalar2=None, op0=ALU.mult,
                    )
                    nc.vector.memset(prod[:, 3:4], 1.0)
                    sout_b = spool.tile([128, 4], F32)  # cols: s_gate, s_up, s_down
                    nc.vector.reciprocal(sout_b[:], prod[:])
                    s_gate, s_up, s_down = (
                        sout_b[:, 0:1], sout_b[:, 1:2], sout_b[:, 2:3])

                    # ============ Phase WQ: quantize + transpose weight shards
                    def quant_transpose_w(raw_tiles, am, gs_ap, gs6_ap, n, tag):
                        """-> wT_sb [128, n//128, 128*len(raw_tiles)] f16.

                        Quantizes in 1024-column chunks to bound work-tile size.
                        """
                        ntile = len(raw_tiles)
                        ncol = n // 128
                        wT_sb = wtr.tile([128, ncol, 128 * ntile], F16, tag=tag,
                                         name=tag)
                        for kk, rt in enumerate(raw_tiles):
                            amap3 = (am[:, kk, :] if ntile > 1 else am[:])
                            for ch in range(n // 1024):
                                wq = wqt.tile([128, 1024], F16, tag="wq")
                                quantize_tile(
                                    nc, workA, rt[:, ch * 1024:(ch + 1) * 1024],
                                    wq[:], 1024, gs_ap, gs6_ap, c448[:],
                                    amax_ap=amap3[:, ch * 64:(ch + 1) * 64],
                                )
                                for jj in range(8):
                                    j = ch * 8 + jj
                                    ps = pst_a.tile([128, 128], F16, tag="pst")
                                    nc.tensor.transpose(
                                        ps[:], wq[:, jj * 128:(jj + 1) * 128],
                                        ident[:])
                                    nc.scalar.activation(
                                        wT_sb[:, j, kk * 128:(kk + 1) * 128],
                                        ps[:], AF.Copy)
                        return wT_sb

                    # gate: quant, stage halves, AG half 0 first
                    gw_sb = quant_transpose_w(gw_raw, gw_am[:], gsw_b[:, 0:1],
                                              gsw6_b[:, 0:1], H, "gw_sb")
                    for k in range(2):
                        nc.sync.dma_start(
                            gwT_loc[k][:, :].rearrange("(c p) i -> p c i", p=128),
                            gw_sb[:, :, k * 256:(k + 1) * 256])
                    nc.gpsimd.collective_compute(
                        "AllGather", ALU.bypass, replica_groups=RG,
                        ins=[gwT_loc[0][:]], outs=[gwT_g[0][:]])
                    uw_sb = quant_transpose_w(uw_raw, uw_am[:], gsw_b[:, 1:2],
                                              gsw6_b[:, 1:2], H, "uw_sb")
                    for k in range(2):
                        nc.sync.dma_start(
                            uwT_loc[k][:, :].rearrange("(c p) i -> p c i", p=128),
                            uw_sb[:, :, k * 256:(k + 1) * 256])
                    nc.gpsimd.collective_compute(
                        "AllGather", ALU.bypass, replica_groups=RG,
                        ins=[uwT_loc[0][:]], outs=[uwT_g[0][:]])
                    nc.gpsimd.collective_compute(
                        "AllGather", ALU.bypass, replica_groups=RG,
                        ins=[gwT_loc[1][:]], outs=[gwT_g[1][:]])
                    nc.gpsimd.collective_compute(
                        "AllGather", ALU.bypass, replica_groups=RG,
                        ins=[uwT_loc[1][:]], outs=[uwT_g[1][:]])
                    # down: rows written in slot order (see module docstring)
                    dw_sb = quant_transpose_w([dw_raw], dw_am, gsw_b[:, 2:3],
                                              gsw6_b[:, 2:3], I, "dw_sb")
                    for a in range(ION):
                        slot = ((a % 4) // 2) * 16 + (a // 4) * 2 + (a % 2)
                        nc.sync.dma_start(
                            dwT_loc[slot * 128:(slot + 1) * 128, :],
                            dw_sb[:, a, :])
                    nc.gpsimd.collective_compute(
                        "AllGather", ALU.bypass, replica_groups=RG,
                        ins=[dwT_loc[:]], outs=[dwT_g[:]])

                # ============ Phase X: x quant + transpose (overlaps AGs)
                with (
                    tc.tile_pool(name="xraw", bufs=2) as xraw,
                    tc.tile_pool(name="xq", bufs=2) as xqp,
                    tc.tile_pool(name="pst_x", bufs=2, space="PSUM") as pst_x,
                    tc.tile_pool(name="workB", bufs=2) as workB,
                ):
                    for tch in range(TCH):
                        xt = xraw.tile([128, H], F32, tag="xraw")
                        nc.sync.dma_start(xt[:], x_in[tch * 128:(tch + 1) * 128, :])
                        xq = xqp.tile([128, H], F16, tag="xq")
                        quantize_tile(nc, workB, xt[:], xq[:], H, gs_x, gs_x6,
                                      c448[:])
                        for hc in range(HC):
                            ps = pst_x.tile([128, 128], F16, tag="pstx")
                            nc.tensor.transpose(
                                ps[:], xq[:, hc * 128:(hc + 1) * 128], ident[:])
                            nc.scalar.activation(
                                xqT[:, hc, tch * 128:(tch + 1) * 128], ps[:],
                                AF.Copy)

                # ============ Phase C: gate/up + SwiGLU + h quant, per I-half
                with (
                    tc.tile_pool(name="wstr", bufs=1) as wstr,
                    tc.tile_pool(name="psgu", bufs=2, space="PSUM") as psgu,
                    tc.tile_pool(name="pst_c", bufs=2, space="PSUM") as pst_c,
                    tc.tile_pool(name="hbuf", bufs=1) as hbuf,
                    tc.tile_pool(name="workC", bufs=1) as workC,
                    tc.tile_pool(name="silp", bufs=3) as silp,
                ):
                    for k in range(2):
                        gwc = wstr.tile([128, HC, IHALF], F16, tag="gwc")
                        uwc = wstr.tile([128, HC, IHALF], F16, tag="uwc")
                        for c in range(NCORES):
                            nc.sync.dma_start(
                                gwc[:, :, c * 256:(c + 1) * 256],
                                gwT_g[k][c * H:(c + 1) * H, :].rearrange(
                                    "(hc p) i -> p hc i", p=128))
                            nc.sync.dma_start(
                                uwc[:, :, c * 256:(c + 1) * 256],
                                uwT_g[k][c * H:(c + 1) * H, :].rearrange(
                                    "(hc p) i -> p hc i", p=128))
                        for tch in range(TCH):
                            htr = hbuf.tile([128, IHALF], F32, tag="htr")
                            for c in range(NCORES):
                                pg = psgu.tile([128, 256], F32, tag="pg")
                                pu = psgu.tile([128, 256], F32, tag="pu")
                                for hc in range(HC):
                                    lhsT = xqT[:, hc, tch * 128:(tch + 1) * 128]
                                    nc.tensor.matmul(
                                        pg[:], lhsT,
                                        gwc[:, hc, c * 256:(c + 1) * 256],
                                        start=(hc == 0), stop=(hc == HC - 1))
                                    nc.tensor.matmul(
                                        pu[:], lhsT,
                                        uwc[:, hc, c * 256:(c + 1) * 256],
                                        start=(hc == 0), stop=(hc == HC - 1))
                                sil = silp.tile([128, 256], F32, tag="sil")
                                nc.scalar.activation(sil[:], pg[:], AF.Silu,
                                                     scale=s_gate)
                                nc.vector.scalar_tensor_tensor(
                                    out=htr[:, c * 256:(c + 1) * 256],
                                    in0=pu[:], scalar=s_up, in1=sil[:],
                                    op0=ALU.mult, op1=ALU.mult)
                            hq = hbuf.tile([128, IHALF], F16, tag="hq")
                            quantize_tile(nc, workC, htr[:], hq[:], IHALF,
                                          gs_h, gs_h6, c448[:])
                            for c in range(NCORES):
                                for jt in range(2):
                                    slot = k * 16 + c * 2 + jt
                                    ps = pst_c.tile([128, 128], F16, tag="pstc")
                                    nc.tensor.transpose(
                                        ps[:],
                                        hq[:, c * 256 + jt * 128:
                                           c * 256 + (jt + 1) * 128],
                                        ident[:])
                                    nc.scalar.activation(
                                        hqT[:, slot, tch * 128:(tch + 1) * 128],
                                        ps[:], AF.Copy)

                # ============ Phase D: down matmul
                with (
                    tc.tile_pool(name="dwcp", bufs=1) as dwcp,
                    tc.tile_pool(name="psd", bufs=2, space="PSUM") as psd,
                    tc.tile_pool(name="obuf", bufs=2) as obuf,
                ):
                    dwc = dwcp.tile([128, ION, H], F16)
                    for r in range(NCORES):
                        nc.sync.dma_start(
                            dwc[:, :, r * 128:(r + 1) * 128],
                            dwT_g[r * I:(r + 1) * I, :].rearrange(
                                "(s p) j -> p s j", p=128))
                    for tch in range(TCH):
                        po = psd.tile([128, H], F32, tag="po")
                        for io in range(ION):
                            lhsT = hqT[:, io, tch * 128:(tch + 1) * 128]
                            nc.tensor.matmul(
                                po[:, 0:512], lhsT, dwc[:, io, 0:512],
                                start=(io == 0), stop=(io == ION - 1))
                            nc.tensor.matmul(
                                po[:, 512:1024], lhsT, dwc[:, io, 512:1024],
                                start=(io == 0), stop=(io == ION - 1))
                        ob = obuf.tile([128, H], F32, tag="ob")
                        nc.scalar.activation(ob[:], po[:], AF.Copy, scale=s_down)
                        nc.sync.dma_start(
                            out_d[tch * 128:(tch + 1) * 128, :], ob[:])

    nc.finalize()
    return nc


# ---------------------------------------------------------------- host side

_PROG = None
_AXON_RUNNER = None
TRACE = False
LAST_EXEC_NS = None
LAST_RESULTS = None


def _get_program():
    global _PROG
    if _PROG is None:
        _PROG = build_program()
    return _PROG


class _AxonRunner:
    """Cached jit shard_map runner (axon only): avoids per-call re-trace,
    host-side concat, and the 32MB zero-output upload of the generic
    run_bass_kernel_spmd path."""

    def __init__(self, nc):
        import jax
        import jax.numpy as jnp
        from jax.sharding import Mesh, PartitionSpec, NamedSharding
        try:
            from jax.experimental.shard_map import shard_map as _sm

            def _shard_map(f, mesh, in_specs, out_specs):
                return _sm(f, mesh=mesh, in_specs=in_specs,
                           out_specs=out_specs, check_rep=False)
        except ImportError:
            from jax import shard_map as _sm2

            def _shard_map(f, mesh, in_specs, out_specs):
                return _sm2(f, mesh=mesh, in_specs=in_specs,
                            out_specs=out_specs, check_vma=False)
        from concourse.bass2jax import (
            _bass_exec_p, install_neuronx_cc_hook, partition_id_tensor,
        )
        install_neuronx_cc_hook()
        pname = nc.partition_id_tensor.name if nc.partition_id_tensor else None
        in_names = ["x_slice", "gw_slice", "uw_slice", "dw_slice", "sc_in",
                    "out_slice"]
        if pname is not None:
            in_names.append(pname)
        out_avals = [jax.core.ShapedArray((T_LOC, H), np.float32)]

        def _body(*args):
            operands = list(args)
            if pname is not None:
                operands.append(partition_id_tensor())
            outs = _bass_exec_p.bind(
                *operands, out_avals=tuple(out_avals), in_names=tuple(in_names),
                out_names=("out_slice",), lowering_input_output_aliases=(),
                sim_require_finite=True, sim_require_nnan=True, nc=nc,
            )
            return tuple(outs)

        devices = jax.devices()[:NCORES]
        mesh = Mesh(np.asarray(devices), ("core",))
        P = PartitionSpec("core")
        self.sharded = jax.jit(
            _shard_map(_body, mesh, (P,) * 6, (P,)),
            donate_argnums=(5,), keep_unused=True,
        )
        self.zero_fn = jax.jit(
            lambda: jnp.zeros((T, H), np.float32),
            out_shardings=NamedSharding(mesh, P),
        )

    def __call__(self, xf, gw, uw, dw, scales):
        r = self.sharded(xf, gw, uw, dw, scales, self.zero_fn())
        return np.asarray(r[0])


def kernel(x, gate_w, up_w, down_w, s_in, s_in_down):
    global _AXON_RUNNER, LAST_EXEC_NS, LAST_RESULTS
    x = np.ascontiguousarray(x, dtype=np.float32)
    gate_w = np.ascontiguousarray(gate_w, dtype=np.float32)
    up_w = np.ascontiguousarray(up_w, dtype=np.float32)
    down_w = np.ascontiguousarray(down_w, dtype=np.float32)
    sc = np.array([[np.asarray(s_in).reshape(-1)[0],
                    np.asarray(s_in_down).reshape(-1)[0]]], dtype=np.float32)
    xf = x.reshape(T, H)
    nc = _get_program()

    if axon_active():
        if _AXON_RUNNER is None:
            _AXON_RUNNER = _AxonRunner(nc)
        scg = np.ascontiguousarray(np.broadcast_to(sc, (NCORES, 2)))
        out = _AXON_RUNNER(xf, gate_w, up_w, down_w, scg)
        LAST_EXEC_NS = None
        return out.reshape(B, S, H)

    in_maps = []
    for c in range(NCORES):
        in_maps.append({
            "x_slice": xf[c * T_LOC:(c + 1) * T_LOC],
            "gw_slice": gate_w[c * I_SH:(c + 1) * I_SH],
            "uw_slice": up_w[c * I_SH:(c + 1) * I_SH],
            "dw_slice": down_w[c * HO_SH:(c + 1) * HO_SH],
            "sc_in": sc,
        })
    res = run_bass_kernel_spmd(
        nc, in_maps, core_ids=list(range(NCORES)), trace=TRACE
    )
    LAST_EXEC_NS = res.exec_time_ns
    LAST_RESULTS = res
    out = np.concatenate([r["out_slice"] for r in res.results], axis=0)
    return out.reshape(B, S, H).astype(np.float32)


if __name__ == "__main__":
    rng = np.random.default_rng(0)
    inputs = dict(
        x=rng.standard_normal((B, S, H), dtype=np.float32),
        gate_w=0.05 * rng.standard_normal((I, H), dtype=np.float32),
        up_w=0.05 * rng.standard_normal((I, H), dtype=np.float32),
        down_w=0.05 * rng.standard_normal((H, I), dtype=np.float32),
        s_in=np.array([700.0], dtype=np.float32),
        s_in_down=np.array([800.0], dtype=np.float32),
    )
    out = kernel(**inputs)
    print("kernel output", out.shape, out.dtype, np.abs(out).max())
